# revision 10
# baseline (speedup 1.0000x reference)
"""Fused cross-attention kernel for Trainium2, 8-way data-parallel over batch.

Low-rank decomposition: S = Q^T K = Jp'^T (Wq' Wk'^T) Jg' = Jp'^T G where
G = M Jg' is computed on host ([65, HW], M = Wq' Wk'^T is 65x65). This cuts
the score matmul contraction from 256 (d) to 65 (c), halving PE work, and
eliminates the Q/K projections entirely.

Value side uses the same trick: O = P V = P (Jg'^T Wv') = (P Jg'^T) Wv' = T Wv'
where T = P Jg'^T is [q, 66] (col 64 = softmax denominator via Jg' ones row,
col 65 = zero pad). Accumulating T costs 66 output columns per k-chunk instead
of 258 for direct P V.

Per core (one batch element):
  for each q-block (512 queries):
    for each k-chunk-pair (2 x 128 keys):
      S^T[k, 2, q] = G-chunk^T Jp'         (PE, bf16, PSUM 2 banks)
      E^T = exp(S^T / 16)                  (ScalarE, one 1024-wide activation)
      T[q, 4, 66] += E^T-subtile^T Jgt     (PE, bf16, 8 x 66-col matmuls)
    T -> SBUF, transpose via PE identity, O[q, 256] = T^T^T Wv'  (PE)
    out[q, :] = O * (1 / T[q, 64])         (DVE)

All matmul operands bf16 (1 cycle/col on PE regardless of width); PSUM
accumulation is fp32. ScalarE (exp) is the pacing engine at ~1us per
1024-element activation; PE runs at ~65% occupancy underneath it.
"""

import sys

sys.path.insert(0, "/opt/trn_rl_repo")

import numpy as np

import concourse.bacc as bacc
import concourse.mybir as mybir
import concourse.tile as tile
from concourse.bass_utils import run_bass_kernel_spmd

B, C, H, W = 8, 64, 64, 64
HW = H * W  # 4096
D = 256
CE = C + 1  # channels + ones row (bias folding)
CT = CE + 1  # T width: 65 channels + zero pad -> 66
N_CORES = 8
QB = 512  # queries per block
N_QB = HW // QB  # 8
N_KC = HW // 128  # 32 key chunks
N_CP = N_KC // 2  # 16 key-chunk pairs
F32 = mybir.dt.float32
BF16 = mybir.dt.bfloat16

_CACHE = {}


GROUPS = [3, 3, 3, 3, 3, 3, 3, 3, 3, 3, 2]  # k-chunks per exp activation (sum 32)


def build_module(reps: int = 1, st_bufs: int = 2, ep_bufs: int = 4, groups=None):
    if groups is None:
        groups = GROUPS
    assert sum(groups) == N_KC
    gmax = max(groups)
    nc = bacc.Bacc("TRN2", target_bir_lowering=False)
    jp_d = nc.dram_tensor("jp", [CE, N_QB, QB], BF16, kind="ExternalInput")
    g_d = nc.dram_tensor("g", [CE, N_KC, 128], BF16, kind="ExternalInput")
    jgt_d = nc.dram_tensor("jgt", [128, N_KC, CT], BF16, kind="ExternalInput")
    wv_d = nc.dram_tensor("wv", [CT, D], BF16, kind="ExternalInput")
    id_d = nc.dram_tensor("ident", [128, 128], F32, kind="ExternalInput")
    out_d = nc.dram_tensor("out", [HW, D], F32, kind="ExternalOutput")

    with tile.TileContext(nc) as tc:
        with tc.tile_pool(name="const", bufs=1) as const:
            jp_t = const.tile([CE, N_QB, QB], BF16, tag="jp")
            g_t = const.tile([CE, N_KC, 128], BF16, tag="g")
            jgt_t = const.tile([128, N_KC, CT], BF16, tag="jgt")
            wv_t = const.tile([CT, D], BF16, tag="wv")
            id_t = const.tile([128, 128], F32, tag="ident")

            # tiny dummy exp issued first: forces the ACT table load to happen
            # during the input-DMA window instead of on the critical path
            dummy = const.tile([1, 2], F32, tag="dummy")
            nc.vector.memset(dummy[:], 0.0)
            nc.scalar.activation(
                dummy[:], dummy[:], mybir.ActivationFunctionType.Exp
            )

            # critical-path inputs first on the sync queue: qb=0 queries and
            # the first score chunks; bulk/epilogue inputs on the gpsimd queue
            nc.sync.dma_start(jp_t[:, 0, :], jp_d[:, 0, :])
            for cq in range(4):
                ks = slice(8 * cq, 8 * cq + 8)
                nc.sync.dma_start(g_t[:, ks, :], g_d[:, ks, :])
            for cq in range(4):
                ks = slice(8 * cq, 8 * cq + 8)
                nc.gpsimd.dma_start(jgt_t[:, ks, :], jgt_d[:, ks, :])
            nc.gpsimd.dma_start(jp_t[:, 1:, :], jp_d[:, 1:, :])
            nc.gpsimd.dma_start(id_t[:], id_d[:])
            nc.gpsimd.dma_start(wv_t[:], wv_d[:])

            with (
                tc.tile_pool(name="stp", bufs=st_bufs, space="PSUM") as stp,
                tc.tile_pool(name="tp", bufs=1, space="PSUM") as tp,
                tc.tile_pool(name="mx", bufs=1, space="PSUM") as mx,
                tc.tile_pool(name="ep", bufs=ep_bufs) as ep,
                tc.tile_pool(name="tsp", bufs=2) as tsp,
                tc.tile_pool(name="ttsp", bufs=2) as ttsp,
                tc.tile_pool(name="outp", bufs=3) as outp,
                tc.tile_pool(name="lp", bufs=4) as lp,
            ):
                for _rep in range(reps):
                    for qb in range(N_QB):
                        t_ps = tp.tile(
                            [128, 4, CT], F32, tag="t", name=f"t_{_rep}_{qb}"
                        )
                        kc0 = 0
                        for gi, gn in enumerate(groups):
                            st = stp.tile(
                                [128, gmax, QB], F32, tag="st", name=f"st_{_rep}_{qb}_{gi}"
                            )
                            for c in range(gn):
                                nc.tensor.matmul(
                                    st[:, c, :],
                                    g_t[:, kc0 + c, :],
                                    jp_t[:, qb, :],
                                )
                            et = ep.tile([128, gmax, QB], BF16, tag="e")
                            nc.scalar.activation(
                                et[:, :gn, :],
                                st[:, :gn, :],
                                mybir.ActivationFunctionType.Exp,
                                scale=1.0 / 16.0,
                            )
                            for c in range(gn):
                                kc = kc0 + c
                                for j in range(4):
                                    # NOTE: start=True resets the WHOLE PSUM
                                    # bank, so only the first matmul of the
                                    # first group may use it — its reset
                                    # zero-fills the other 3 groups' regions.
                                    nc.tensor.matmul(
                                        t_ps[:, j, :],
                                        et[:, c, j * 128 : (j + 1) * 128],
                                        jgt_t[:, kc, :],
                                        start=(kc == 0 and j == 0),
                                        stop=(kc == N_KC - 1),
                                        skip_group_check=True,
                                    )
                            kc0 += gn
                        # epilogue: transpose T and project through Wv'
                        t_sb = tsp.tile([128, 4, CT], F32, tag="ts", name=f"ts_{_rep}_{qb}")
                        nc.vector.tensor_copy(t_sb[:], t_ps[:])
                        tt_ps = mx.tile(
                            [128, 4, 128], F32, tag="x", name=f"tt_{_rep}_{qb}"
                        )
                        for j in range(4):
                            nc.tensor.matmul(
                                tt_ps[:CT, j, :],
                                t_sb[:, j, :],
                                id_t[:],
                                is_transpose=True,
                                start=(j == 0),
                                stop=(j == 3),
                                skip_group_check=True,
                            )
                        tt_sb = ttsp.tile([128, 4, 128], BF16, tag="tts", name=f"tts_{_rep}_{qb}")
                        nc.vector.tensor_copy(tt_sb[:CT, :, :], tt_ps[:CT, :, :])
                        for j in range(4):
                            row = qb * 4 + j
                            o_ps = mx.tile(
                                [128, D],
                                F32,
                                tag="x",
                                name=f"o_{_rep}_{qb}_{j}",
                                padded_shape=[128, 4 * 128],
                            )
                            nc.tensor.matmul(
                                o_ps[:], tt_sb[:CT, j, :], wv_t[:]
                            )
                            linv = lp.tile([128, 1], F32, tag="l")
                            nc.vector.reciprocal(linv[:], t_sb[:, j, C : C + 1])
                            ot = outp.tile([128, D], F32, tag="ot")
                            nc.vector.tensor_scalar_mul(ot[:], o_ps[:], linv[:])
                            nc.sync.dma_start(
                                out_d[row * 128 : (row + 1) * 128, :], ot[:]
                            )

    nc.compile()
    return nc


def _get_module(reps: int = 1, **kw):
    key = (reps, tuple(sorted(kw.items())))
    if key not in _CACHE:
        _CACHE[key] = build_module(reps, **kw)
    return _CACHE[key]


_ROW1 = np.ones((1, HW), np.float32)
_EYE = np.eye(128, dtype=np.float32)


def _prep_in_maps(inputs):
    import ml_dtypes

    bf = ml_dtypes.bfloat16
    jp = np.asarray(inputs["Jp_embedding"], np.float32).reshape(B, C, HW)
    jg = np.asarray(inputs["Jg_embedding"], np.float32).reshape(B, C, HW)
    wq = np.concatenate(
        [np.asarray(inputs["Wq"], np.float32).T, np.asarray(inputs["bq"], np.float32)[None, :]], 0
    )
    wk = np.concatenate(
        [np.asarray(inputs["Wk"], np.float32).T, np.asarray(inputs["bk"], np.float32)[None, :]], 0
    )
    wv = np.concatenate(
        [
            np.asarray(inputs["Wv"], np.float32).T,
            np.asarray(inputs["bv"], np.float32)[None, :],
            np.zeros((1, D), np.float32),
        ],
        0,
    )
    m = wq @ wk.T  # [65, 65]
    ident = _EYE
    wv_b = wv.astype(bf)
    maps = []
    for b in range(B):
        jp_b = np.concatenate([jp[b], _ROW1], 0)  # [65, HW]
        jg_b = np.concatenate([jg[b], _ROW1], 0)
        g_b = m @ jg_b  # [65, HW]
        jgt_b = np.concatenate([jg_b.T, np.zeros((HW, 1), np.float32)], 1)  # [HW, 66]
        maps.append(
            {
                "jp": jp_b.reshape(CE, N_QB, QB).astype(bf),
                "g": g_b.reshape(CE, N_KC, 128).astype(bf),
                "jgt": np.ascontiguousarray(
                    jgt_b.reshape(N_KC, 128, CT).transpose(1, 0, 2)
                ).astype(bf),
                "wv": wv_b,
                "ident": ident,
            }
        )
    return maps


def kernel(**inputs):
    nc = _get_module()
    in_maps = _prep_in_maps(inputs)
    res = run_bass_kernel_spmd(nc, in_maps, core_ids=list(range(N_CORES)))
    return np.stack(
        [res.results[b]["out"].reshape(D, H, W) for b in range(B)], axis=0
    )
